# revision 11
# baseline (speedup 1.0000x reference)
"""Self-contained Trainium2 Bass kernel for the "Attentive" GNN message-passing
problem:

    x: [8192, 256] f32, attn_vectors: [4, 256] f32
    e_h = l2_normalize(attn_vectors[h] * x, axis=-1)        # [H, N, D]
    out = (1/H) sum_h e_h @ e_h^T                           # [N, N]

Strategy (8 NeuronCores, SPMD, no collectives):
  - The output is SYMMETRIC: only the 136 upper-triangle 512x512 blocks of
    the 16x16 block grid are computed; the host mirrors the rest.
  - Blocks are dealt with a rotation scheme: a FIXED set S of 17 slot-pairs
    covers all 136 unordered pairs exactly once under slot -> slot+c (mod 16),
    c = core id. Every core runs the IDENTICAL program on x rolled by
    c*512 rows (host-side roll), so the program is core-agnostic.
  - Every core builds all 16 normalized/scaled panels g_p resident in SBUF:
       g[d_chunk, kc, n] = SCALE_A * attn_h[d] * x[n, d] * rnorm_h[n]
    (kc = h*2+c chunks of 128 contraction rows), then computes its 17
    blocks as plain g_i^T g_j matmuls.
  - fp8e4 (e4m3) matmuls in DoubleRow perf mode (two 128-deep k-tiles per
    instruction). g is scaled 16x up (SCALE_A=8 vs the exact 0.5) so fp8
    values sit in the normal range; the host divides the result by 256.
  - Output blocks are DMA'd DIRECTLY from PSUM to DRAM (no SBUF staging
    copies), two 128-row groups at a time.
  - Row norms: transposed-PE matmuls xsq^T @ attn^2 per panel; rnorm rows
    bounce through DRAM (bf16) and return as one broadcast DMA per panel.
  - x is passed as bf16 from the host (halves the input HBM traffic; the
    bf16 rounding is shared by the norm and the matmul, so rows stay
    exactly unit-norm).
"""

from contextlib import ExitStack

import numpy as np

N, D, H = 8192, 256, 4
NCORES = 8
P = 128
PANEL = 512
NPANELS = N // PANEL  # 16
KCH = (H * D) // P  # 8 contraction chunks of 128
CHD = D // P  # 2 chunks per head
SUB = PANEL // P  # 4 row sub-blocks per panel
EPS = 1e-12

USE_FP8 = True
DIRECT_PSUM_DMA = False

SCALE_A = 8.0 if USE_FP8 else 0.5  # folded into a8 input
OUT_SCALE = (0.5 / SCALE_A) ** 2  # host-side (or staged-copy) factor

# Fixed slot-pair set: covers all 136 unordered panel pairs exactly once
# under (si, sj) -> (si+c, sj+c) mod 16, c = 0..7.
S_PAIRS = (
    [(0, 0)]
    + [(0, d) for d in range(1, 9)]
    + [(8, 8)]
    + [(8, 8 + d) for d in range(1, 8)]
)
S_SORTED = sorted(S_PAIRS, key=lambda s: (max(s), min(s)))
NBLK = len(S_SORTED)  # 17

_COMPILED = {}


def _build_bass():
    import concourse.bass as bass
    import concourse.tile as tile
    from concourse import bacc, mybir

    f32 = mybir.dt.float32
    bf16 = mybir.dt.bfloat16
    fp8 = mybir.dt.float8e4
    gdt = fp8 if USE_FP8 else bf16

    nc = bacc.Bacc(
        "TRN2",
        target_bir_lowering=False,
        debug=False,
        enable_asserts=False,
        num_devices=NCORES,
    )
    x_t = nc.dram_tensor("x", [N, D], bf16, kind="ExternalInput")
    # Host-precomputed functions of attn_vectors (tiny):
    #   w_sq[d, c*H+h] = attn[h, c*128+d]^2          (bf16, norm matmul rhs)
    #   a8[d, kc]      = SCALE_A*attn[h, c*128+d]    (f32, kc = h*2+c)
    ws_t = nc.dram_tensor("w_sq", [P, CHD * H], bf16, kind="ExternalInput")
    a8_t = nc.dram_tensor("a8", [P, KCH], f32, kind="ExternalInput")
    out_t = nc.dram_tensor("out", [NBLK * PANEL, PANEL], f32, kind="ExternalOutput")

    x, out = x_t.ap(), out_t.ap()

    with tile.TileContext(nc) as tc, ExitStack() as ctx:
        consts = ctx.enter_context(tc.tile_pool(name="consts", bufs=1))
        loads = ctx.enter_context(tc.tile_pool(name="loads", bufs=4))
        gpool = ctx.enter_context(tc.tile_pool(name="gpool", bufs=1))
        gstage = ctx.enter_context(tc.tile_pool(name="gstage", bufs=2))
        xtp = ctx.enter_context(tc.tile_pool(name="xtp", bufs=3))
        sq = ctx.enter_context(tc.tile_pool(name="sq", bufs=2))
        small = ctx.enter_context(tc.tile_pool(name="small", bufs=3))
        bcp = ctx.enter_context(tc.tile_pool(name="bcp", bufs=3))
        outp = ctx.enter_context(tc.tile_pool(name="outp", bufs=2))
        dram = ctx.enter_context(tc.tile_pool(name="dram", bufs=1, space="DRAM"))
        ps_tp = ctx.enter_context(tc.tile_pool(name="ps_tp", bufs=2, space="PSUM"))
        ps_nm = ctx.enter_context(tc.tile_pool(name="ps_nm", bufs=1, space="PSUM"))
        ps_out = ctx.enter_context(tc.tile_pool(name="ps_out", bufs=2, space="PSUM"))

        from concourse.masks import make_identity

        w_sq = consts.tile([P, CHD * H], bf16)
        nc.sync.dma_start(w_sq[:], ws_t.ap()[:])
        a8 = consts.tile([P, KCH], f32)
        nc.sync.dma_start(a8[:], a8_t.ap()[:])
        ident = consts.tile([P, P], f32)
        make_identity(nc, ident[:])
        identb = consts.tile([P, P], bf16)
        make_identity(nc, identb[:])

        gtiles = []  # resident per-panel g (built lazily)
        bcs = {}
        xTs = {}
        xloads = {}

        def load_panel(p):
            xl = loads.tile([P, SUB, D], bf16, tag="xload")
            nc.sync.dma_start(
                xl[:], x[p * PANEL : (p + 1) * PANEL, :].rearrange("(i q) d -> q i d", q=P)
            )
            xloads[p] = xl

        def prepass_front(p):
            """Transpose panel p, compute its rnorm, park it in DRAM, and
            start the broadcast DMA back into bcs[p]."""
            xl = xloads.pop(p)
            xT = xtp.tile([P, CHD, PANEL], bf16, tag="xT")
            for c in range(CHD):
                tp = ps_tp.tile([P, PANEL], bf16, tag="tp")
                for i in range(SUB):
                    nc.tensor.transpose(
                        tp[:, i * P : (i + 1) * P],
                        xl[:, i, c * P : (c + 1) * P],
                        identb[:],
                    )
                nc.vector.tensor_copy(xT[:, c, :], tp[:])
            xTs[p] = xT
            xsq = sq.tile([P, CHD, PANEL], bf16, tag="xsq")
            nc.vector.tensor_mul(xsq[:], xT[:], xT[:])
            pn = ps_nm.tile([P, SUB * H], f32, tag="pn")
            for i in range(SUB):
                for c in range(CHD):
                    nc.tensor.matmul(
                        pn[:, i * H : (i + 1) * H],
                        xsq[:, c, i * P : (i + 1) * P],
                        w_sq[:, c * H : (c + 1) * H],
                        start=(c == 0),
                        stop=(c == CHD - 1),
                    )
            # eps-clamp; the input AP also permutes [q,(i h)] -> [q,(h i)]
            # so that after the PE transpose the flat DRAM tile is h-major.
            clamped = small.tile([P, SUB * H], f32, tag="clamped")
            nc.vector.tensor_scalar_max(
                clamped[:], pn[:].rearrange("q (i h) -> q h i", h=H), EPS
            )
            root = small.tile([P, SUB * H], f32, tag="root")
            nc.scalar.sqrt(root[:], clamped[:])
            rnorm = small.tile([P, SUB * H], f32, tag="rnorm")
            nc.vector.reciprocal(rnorm[:], root[:])
            pt = ps_nm.tile([SUB * H, P], f32, tag="pt")
            nc.tensor.transpose(pt[:], rnorm[:], ident[:])
            rno = small.tile([SUB * H, P], bf16, tag="rno")
            nc.vector.tensor_copy(rno[:], pt[:])
            rnd = dram.tile([SUB * H, P], bf16, name=f"rnd{p}")
            nc.sync.dma_start(rnd[:], rno[:])
            # broadcast back: bc[q, h, n] = rnorm_h[n] for all q
            bc = bcp.tile([P, H, PANEL], bf16, tag="bc")
            src = bass.AP(rnd.tensor, rnd.offset, [[0, P], [PANEL, H], [1, PANEL]])
            nc.sync.dma_start(bc[:], src)
            bcs[p] = bc

        def g_build(p):
            """gb[:, kc, :] = (xT[:, c, :] * a8[:, kc]) * bc[:, h, :] in bf16
            (DVE 4x fast mode needs all-16-bit operands), then one casting
            SWDGE DMA (gpsimd) converts the panel to resident fp8."""
            g = gpool.tile([P, KCH, PANEL], gdt, name=f"g{p}")
            gtiles.append(g)
            assert len(gtiles) == p + 1
            xT, bc = xTs.pop(p), bcs.pop(p)
            if USE_FP8:
                gb = gstage.tile([P, KCH, PANEL], bf16, tag="gb")
            else:
                gb = g
            for kc in range(KCH):
                h, c = divmod(kc, CHD)
                nc.vector.scalar_tensor_tensor(
                    gb[:, kc, :],
                    xT[:, c, :],
                    a8[:, kc : kc + 1],
                    bc[:, h, :],
                    mybir.AluOpType.mult,
                    mybir.AluOpType.mult,
                )
            if USE_FP8:
                nc.gpsimd.dma_start(g[:], gb[:])

        def do_block(b):
            si, sj = S_SORTED[b]
            gi, gj = gtiles[si], gtiles[sj]
            for u in range(2):  # two 256-row halves of the 512-row block
                acc = ps_out.tile([P, 2, PANEL], f32, tag="acc")
                for r2 in range(2):
                    r = 2 * u + r2
                    if USE_FP8:
                        for kp in range(KCH // 2):
                            nc.tensor.matmul(
                                acc[:, r2, :],
                                gi[:, 2 * kp : 2 * kp + 2, r * P : (r + 1) * P],
                                gj[:, 2 * kp : 2 * kp + 2, :],
                                start=(kp == 0),
                                stop=(kp == KCH // 2 - 1),
                                perf_mode=mybir.MatmulPerfMode.DoubleRow,
                            )
                    else:
                        for kc in range(KCH):
                            nc.tensor.matmul(
                                acc[:, r2, :],
                                gi[:, kc, r * P : (r + 1) * P],
                                gj[:, kc, :],
                                start=(kc == 0),
                                stop=(kc == KCH - 1),
                            )
                dst = out[
                    b * PANEL + u * 2 * P : b * PANEL + (u + 1) * 2 * P, :
                ].rearrange("(r q) c -> q r c", q=P)
                if DIRECT_PSUM_DMA:
                    nc.sync.dma_start(dst, acc[:])
                else:
                    ot = outp.tile([P, 2, PANEL], f32, tag="ot")
                    nc.scalar.mul(ot[:], acc[:], OUT_SCALE)
                    nc.sync.dma_start(dst, ot[:])

        blocks_at = {}
        for b, (si, sj) in enumerate(S_SORTED):
            blocks_at.setdefault(max(si, sj), []).append(b)

        # software pipeline: loads 3 ahead, g-build 1 behind, blocks 2 behind
        for p in range(3):
            load_panel(p)
        for p in range(NPANELS):
            if p + 3 < NPANELS:
                load_panel(p + 3)
            prepass_front(p)
            if p >= 1:
                g_build(p - 1)
            if p >= 2:
                for b in blocks_at.get(p - 2, []):
                    do_block(b)
        g_build(NPANELS - 1)
        for b in blocks_at.get(NPANELS - 2, []):
            do_block(b)
        for b in blocks_at.get(NPANELS - 1, []):
            do_block(b)

    nc.compile()
    return nc


def _get_compiled():
    if "nc" not in _COMPILED:
        _COMPILED["nc"] = _build_bass()
    return _COMPILED["nc"]


def host_side_inputs(x, attn):
    """Per-core input maps. Core c sees x rolled up by c*512 rows so the
    identical program computes a distinct set of output blocks."""
    import ml_dtypes

    w_sq = np.zeros((P, CHD * H), dtype=np.float32)
    a8 = np.zeros((P, KCH), dtype=np.float32)
    for c in range(CHD):
        w_sq[:, c * H : (c + 1) * H] = (attn[:, c * P : (c + 1) * P] ** 2).T
    for kc in range(KCH):
        h, c = divmod(kc, CHD)
        a8[:, kc] = SCALE_A * attn[h, c * P : (c + 1) * P]
    w_sq = w_sq.astype(ml_dtypes.bfloat16)
    xb = x.astype(ml_dtypes.bfloat16)
    return [
        {
            "x": np.ascontiguousarray(np.roll(xb, -c * PANEL, axis=0)),
            "w_sq": w_sq,
            "a8": a8,
        }
        for c in range(NCORES)
    ]


def assemble(results):
    """Scatter each core's 17 blocks (and their mirrors) into the full
    [N, N] output."""
    scale = OUT_SCALE if DIRECT_PSUM_DMA else 1.0
    full = np.empty((N, N), dtype=np.float32)
    for c in range(NCORES):
        o = np.asarray(results[c]["out"], dtype=np.float32)
        for b, (si, sj) in enumerate(S_SORTED):
            bi, bj = (si + c) % NPANELS, (sj + c) % NPANELS
            blk = o[b * PANEL : (b + 1) * PANEL, :]
            if scale != 1.0:
                blk = blk * scale
            if bi == bj:
                sblk = (blk + blk.T) * 0.5
                # l2-normalized rows: the diagonal is exactly 1
                np.fill_diagonal(sblk, 1.0)
                full[bi * PANEL : (bi + 1) * PANEL, bj * PANEL : (bj + 1) * PANEL] = (
                    sblk
                )
            else:
                full[bi * PANEL : (bi + 1) * PANEL, bj * PANEL : (bj + 1) * PANEL] = blk
                full[bj * PANEL : (bj + 1) * PANEL, bi * PANEL : (bi + 1) * PANEL] = (
                    blk.T
                )
    return full


def kernel(**inputs) -> np.ndarray:
    from concourse import bass_utils

    x = np.ascontiguousarray(np.asarray(inputs["x"], dtype=np.float32))
    attn = np.ascontiguousarray(np.asarray(inputs["attn_vectors"], dtype=np.float32))
    nc = _get_compiled()
    res = bass_utils.run_bass_kernel_spmd(
        nc, host_side_inputs(x, attn), core_ids=list(range(NCORES))
    )
    return assemble(res.results)


# revision 17
# speedup vs baseline: 1.1645x; 1.1645x over previous
"""Self-contained Trainium2 Bass kernel for the "Attentive" GNN message-passing
problem:

    x: [8192, 256] f32, attn_vectors: [4, 256] f32
    e_h = l2_normalize(attn_vectors[h] * x, axis=-1)        # [H, N, D]
    out = (1/H) sum_h e_h @ e_h^T                           # [N, N]

Strategy (8 NeuronCores, SPMD, no collectives):
  - The output is SYMMETRIC: only the 136 upper-triangle 512x512 blocks of
    the 16x16 block grid are computed; the host mirrors the rest.
  - Blocks are dealt with a rotation scheme: a FIXED set S of 17 slot-pairs
    covers all 136 unordered pairs exactly once under slot -> slot+c (mod 16),
    c = core id. Every core runs the IDENTICAL program on x rolled by
    c*512 rows (host-side roll), so the program is core-agnostic.
  - Every core builds all 16 normalized/scaled panels g_p resident in SBUF:
       g[d_chunk, kc, n] = SCALE_A * attn_h[d] * x[n, d] * rnorm_h[n]
    (kc = h*2+c chunks of 128 contraction rows), then computes its 17
    blocks as plain g_i^T g_j matmuls.
  - fp8e4 (e4m3) matmuls in DoubleRow perf mode (two 128-deep k-tiles per
    instruction). g is scaled 16x up (SCALE_A=8 vs the exact 0.5) so fp8
    values sit in the normal range; the host divides the result by 256.
  - Output blocks are DMA'd DIRECTLY from PSUM to DRAM (no SBUF staging
    copies), two 128-row groups at a time.
  - Row norms: transposed-PE matmuls xsq^T @ attn^2 per panel; rnorm rows
    bounce through DRAM (bf16) and return as one broadcast DMA per panel.
  - x is passed as bf16 from the host (halves the input HBM traffic; the
    bf16 rounding is shared by the norm and the matmul, so rows stay
    exactly unit-norm).
"""

from contextlib import ExitStack

import numpy as np

N, D, H = 8192, 256, 4
NCORES = 8
P = 128
PANEL = 512
NPANELS = N // PANEL  # 16
KCH = (H * D) // P  # 8 contraction chunks of 128
CHD = D // P  # 2 chunks per head
SUB = PANEL // P  # 4 row sub-blocks per panel
EPS = 1e-12

USE_FP8 = True
DIRECT_PSUM_DMA = False

SCALE_A = 8.0 if USE_FP8 else 0.5  # folded into a8 input
OUT_SCALE = (0.5 / SCALE_A) ** 2  # host-side (or staged-copy) factor

# Fixed slot-pair set: covers all 136 unordered panel pairs exactly once
# under (si, sj) -> (si+c, sj+c) mod 16, c = 0..7.
S_PAIRS = (
    [(0, 0)]
    + [(0, d) for d in range(1, 9)]
    + [(8, 8)]
    + [(8, 8 + d) for d in range(1, 8)]
)
S_SORTED = sorted(S_PAIRS, key=lambda s: (max(s), min(s)))
NBLK = len(S_SORTED)  # 17

_COMPILED = {}


def _build_bass():
    import concourse.bass as bass
    import concourse.tile as tile
    from concourse import bacc, mybir

    f32 = mybir.dt.float32
    bf16 = mybir.dt.bfloat16
    fp8 = mybir.dt.float8e4
    gdt = fp8 if USE_FP8 else bf16

    nc = bacc.Bacc(
        "TRN2",
        target_bir_lowering=False,
        debug=False,
        enable_asserts=False,
        num_devices=NCORES,
    )
    x_t = nc.dram_tensor("x", [N, D], bf16, kind="ExternalInput")
    # Host-precomputed functions of attn_vectors (tiny):
    #   w_sq[d, c*H+h] = attn[h, c*128+d]^2          (bf16, norm matmul rhs)
    #   a8[d, kc]      = SCALE_A*attn[h, c*128+d]    (f32, kc = h*2+c)
    ws_t = nc.dram_tensor("w_sq", [P, CHD * H], bf16, kind="ExternalInput")
    a8_t = nc.dram_tensor("a8", [P, KCH], f32, kind="ExternalInput")
    out_t = nc.dram_tensor("out", [NBLK * PANEL, PANEL], f32, kind="ExternalOutput")

    x, out = x_t.ap(), out_t.ap()

    with tile.TileContext(nc) as tc, ExitStack() as ctx:
        consts = ctx.enter_context(tc.tile_pool(name="consts", bufs=1))
        loads = ctx.enter_context(tc.tile_pool(name="loads", bufs=4))
        gpool = ctx.enter_context(tc.tile_pool(name="gpool", bufs=1))
        gstage = ctx.enter_context(tc.tile_pool(name="gstage", bufs=2))
        axp = ctx.enter_context(tc.tile_pool(name="axp", bufs=2))
        xtp = ctx.enter_context(tc.tile_pool(name="xtp", bufs=3))
        sq = ctx.enter_context(tc.tile_pool(name="sq", bufs=2))
        small = ctx.enter_context(tc.tile_pool(name="small", bufs=3))
        bcp = ctx.enter_context(tc.tile_pool(name="bcp", bufs=3))
        outp = ctx.enter_context(tc.tile_pool(name="outp", bufs=2))
        dram = ctx.enter_context(tc.tile_pool(name="dram", bufs=1, space="DRAM"))
        ps_tp = ctx.enter_context(tc.tile_pool(name="ps_tp", bufs=2, space="PSUM"))
        ps_nm = ctx.enter_context(tc.tile_pool(name="ps_nm", bufs=1, space="PSUM"))
        ps_out = ctx.enter_context(tc.tile_pool(name="ps_out", bufs=2, space="PSUM"))

        from concourse.masks import make_identity

        w_sq = consts.tile([P, CHD * H], bf16)
        nc.sync.dma_start(w_sq[:], ws_t.ap()[:])
        a8 = consts.tile([P, KCH], f32)
        nc.sync.dma_start(a8[:], a8_t.ap()[:])
        ident = consts.tile([P, P], f32)
        make_identity(nc, ident[:])
        identb = consts.tile([P, P], bf16)
        make_identity(nc, identb[:])

        gtiles = []  # resident per-panel g (built lazily)
        bcs = {}
        xTs = {}
        xloads = {}

        def load_panel(p):
            xl = loads.tile([P, SUB, D], bf16, tag="xload")
            nc.sync.dma_start(
                xl[:], x[p * PANEL : (p + 1) * PANEL, :].rearrange("(i q) d -> q i d", q=P)
            )
            xloads[p] = xl

        def prepass_front(p):
            """Transpose panel p, compute its rnorm, park it in DRAM, and
            start the broadcast DMA back into bcs[p]."""
            xl = xloads.pop(p)
            xT = xtp.tile([P, CHD, PANEL], bf16, tag="xT")
            for c in range(CHD):
                tp = ps_tp.tile([P, PANEL], bf16, tag="tp")
                for i in range(SUB):
                    nc.tensor.transpose(
                        tp[:, i * P : (i + 1) * P],
                        xl[:, i, c * P : (c + 1) * P],
                        identb[:],
                    )
                nc.scalar.copy(xT[:, c, :], tp[:])
            xTs[p] = xT
            xsq = sq.tile([P, CHD, PANEL], bf16, tag="xsq")
            nc.scalar.square(xsq[:], xT[:])
            pn = ps_nm.tile([P, SUB * H], f32, tag="pn")
            for i in range(SUB):
                for c in range(CHD):
                    nc.tensor.matmul(
                        pn[:, i * H : (i + 1) * H],
                        xsq[:, c, i * P : (i + 1) * P],
                        w_sq[:, c * H : (c + 1) * H],
                        start=(c == 0),
                        stop=(c == CHD - 1),
                    )
            # eps-clamp; the input AP also permutes [q,(i h)] -> [q,(h i)]
            # so that after the PE transpose the flat DRAM tile is h-major.
            clamped = small.tile([P, SUB * H], f32, tag="clamped")
            nc.vector.tensor_scalar_max(
                clamped[:], pn[:].rearrange("q (i h) -> q h i", h=H), EPS
            )
            root = small.tile([P, SUB * H], f32, tag="root")
            nc.scalar.sqrt(root[:], clamped[:])
            rnorm = small.tile([P, SUB * H], f32, tag="rnorm")
            nc.vector.reciprocal(rnorm[:], root[:])
            pt = ps_nm.tile([SUB * H, P], f32, tag="pt")
            nc.tensor.transpose(pt[:], rnorm[:], ident[:])
            rno = small.tile([SUB * H, P], bf16, tag="rno")
            nc.vector.tensor_copy(rno[:], pt[:])
            rnd = dram.tile([SUB * H, P], bf16, name=f"rnd{p}")
            nc.gpsimd.dma_start(rnd[:], rno[:])
            # broadcast back: bc[q, h, n] = rnorm_h[n] for all q
            bc = bcp.tile([P, H, PANEL], bf16, tag="bc")
            src = bass.AP(rnd.tensor, rnd.offset, [[0, P], [PANEL, H], [1, PANEL]])
            nc.gpsimd.dma_start(bc[:], src)
            bcs[p] = bc

        def g_build(p):
            """gb[:, kc, :] = (xT[:, c, :] * a8[:, kc]) * bc[:, h, :] in bf16
            (DVE 4x fast mode needs all-16-bit operands), then one casting
            SWDGE DMA (gpsimd) converts the panel to resident fp8."""
            g = gpool.tile([P, KCH, PANEL], gdt, name=f"g{p}")
            gtiles.append(g)
            assert len(gtiles) == p + 1
            xT, bc = xTs.pop(p), bcs.pop(p)
            if USE_FP8:
                gb = gstage.tile([P, KCH, PANEL], bf16, tag="gb")
            else:
                gb = g
            # axT[:, kc, :] = xT[:, c, :] * a8[:, kc] — tensor_scalar runs the
            # DVE 4x fast path; the fused scalar_tensor_tensor does not.
            axT = axp.tile([P, KCH, PANEL], bf16, tag="axT")
            for kc in range(KCH):
                h, c = divmod(kc, CHD)
                nc.vector.tensor_scalar_mul(
                    axT[:, kc, :], xT[:, c, :], a8[:, kc : kc + 1]
                )
            # gb = axT * bc (h-broadcast over the c sub-chunks), one wide op
            in1 = bass.AP(
                bc.tensor,
                bc.offset,
                [list(bc.ap[0]), [PANEL, H], [0, CHD], [1, PANEL]],
            )
            nc.vector.tensor_tensor(
                gb[:].rearrange("q (h c) n -> q h c n", h=H),
                axT[:].rearrange("q (h c) n -> q h c n", h=H),
                in1,
                mybir.AluOpType.mult,
            )
            if USE_FP8:
                nc.gpsimd.dma_start(g[:], gb[:])

        def do_block(b):
            si, sj = S_SORTED[b]
            gi, gj = gtiles[si], gtiles[sj]
            ot = outp.tile([P, SUB, PANEL], f32, tag="ot")
            for u in range(2):  # two 256-row halves of the 512-row block
                acc = ps_out.tile([P, 2, PANEL], f32, tag="acc")
                for r2 in range(2):
                    r = 2 * u + r2
                    if USE_FP8:
                        for kp in range(KCH // 2):
                            nc.tensor.matmul(
                                acc[:, r2, :],
                                gi[:, 2 * kp : 2 * kp + 2, r * P : (r + 1) * P],
                                gj[:, 2 * kp : 2 * kp + 2, :],
                                start=(kp == 0),
                                stop=(kp == KCH // 2 - 1),
                                perf_mode=mybir.MatmulPerfMode.DoubleRow,
                            )
                    else:
                        for kc in range(KCH):
                            nc.tensor.matmul(
                                acc[:, r2, :],
                                gi[:, kc, r * P : (r + 1) * P],
                                gj[:, kc, :],
                                start=(kc == 0),
                                stop=(kc == KCH - 1),
                            )
                nc.scalar.mul(ot[:, 2 * u : 2 * u + 2, :], acc[:], OUT_SCALE)
            dst = out[b * PANEL : (b + 1) * PANEL, :].rearrange(
                "(r q) c -> q r c", q=P
            )
            nc.sync.dma_start(dst, ot[:])

        blocks_at = {}
        for b, (si, sj) in enumerate(S_SORTED):
            blocks_at.setdefault(max(si, sj), []).append(b)

        # software pipeline: loads 3 ahead, g-build 1 behind, blocks 2 behind
        for p in range(3):
            load_panel(p)
        for p in range(NPANELS):
            if p + 3 < NPANELS:
                load_panel(p + 3)
            prepass_front(p)
            if p >= 1:
                g_build(p - 1)
            if p >= 2:
                for b in blocks_at.get(p - 2, []):
                    do_block(b)
        g_build(NPANELS - 1)
        for b in blocks_at.get(NPANELS - 2, []):
            do_block(b)
        for b in blocks_at.get(NPANELS - 1, []):
            do_block(b)

    nc.compile()
    return nc


def _get_compiled():
    if "nc" not in _COMPILED:
        _COMPILED["nc"] = _build_bass()
    return _COMPILED["nc"]


def host_side_inputs(x, attn):
    """Per-core input maps. Core c sees x rolled up by c*512 rows so the
    identical program computes a distinct set of output blocks."""
    import ml_dtypes

    w_sq = np.zeros((P, CHD * H), dtype=np.float32)
    a8 = np.zeros((P, KCH), dtype=np.float32)
    for c in range(CHD):
        w_sq[:, c * H : (c + 1) * H] = (attn[:, c * P : (c + 1) * P] ** 2).T
    for kc in range(KCH):
        h, c = divmod(kc, CHD)
        a8[:, kc] = SCALE_A * attn[h, c * P : (c + 1) * P]
    w_sq = w_sq.astype(ml_dtypes.bfloat16)
    xb = x.astype(ml_dtypes.bfloat16)
    return [
        {
            "x": np.ascontiguousarray(np.roll(xb, -c * PANEL, axis=0)),
            "w_sq": w_sq,
            "a8": a8,
        }
        for c in range(NCORES)
    ]


def assemble(results):
    """Scatter each core's 17 blocks (and their mirrors) into the full
    [N, N] output."""
    scale = OUT_SCALE if DIRECT_PSUM_DMA else 1.0
    full = np.empty((N, N), dtype=np.float32)
    for c in range(NCORES):
        o = np.asarray(results[c]["out"], dtype=np.float32)
        for b, (si, sj) in enumerate(S_SORTED):
            bi, bj = (si + c) % NPANELS, (sj + c) % NPANELS
            blk = o[b * PANEL : (b + 1) * PANEL, :]
            if scale != 1.0:
                blk = blk * scale
            if bi == bj:
                sblk = (blk + blk.T) * 0.5
                # l2-normalized rows: the diagonal is exactly 1
                np.fill_diagonal(sblk, 1.0)
                full[bi * PANEL : (bi + 1) * PANEL, bj * PANEL : (bj + 1) * PANEL] = (
                    sblk
                )
            else:
                full[bi * PANEL : (bi + 1) * PANEL, bj * PANEL : (bj + 1) * PANEL] = blk
                full[bj * PANEL : (bj + 1) * PANEL, bi * PANEL : (bi + 1) * PANEL] = (
                    blk.T
                )
    return full


def kernel(**inputs) -> np.ndarray:
    from concourse import bass_utils

    x = np.ascontiguousarray(np.asarray(inputs["x"], dtype=np.float32))
    attn = np.ascontiguousarray(np.asarray(inputs["attn_vectors"], dtype=np.float32))
    nc = _get_compiled()
    res = bass_utils.run_bass_kernel_spmd(
        nc, host_side_inputs(x, attn), core_ids=list(range(NCORES))
    )
    return assemble(res.results)
